# revision 4
# baseline (speedup 1.0000x reference)
"""Bahdanau-attention Trainium2 kernel (Bass/Tile, 8-core data-parallel SPMD).

reference math (per batch b):
    proj_f = features @ W1 + b1                 [T, U]
    proj_h = hidden @ W2 + b2                   [U]
    score  = tanh(proj_f + proj_h)              [T, U]
    logits = score @ V (+ bv, softmax-invariant)[T, 1]
    attn   = softmax(logits, axis=T)            [T, 1]
    context= sum_t attn * features              [D]

Sharding: batch dim B=32 split 4-per-core across 8 cores; weights replicated.

Per-core plan (BL=4, T=2048, D=U=512, P=128):
  - load F natural [128t, 512d] tiles (kept in SBUF for the context pass)
  - PE-transpose F tiles -> FT [128d, t] (fp32 transpose-mode matmuls)
  - main matmul (float32r, 1 cyc/row): psum[u,t] = sum_k W1[k,uc].T @ FT[k]
  - ACT tanh with per-partition bias = (hidden@W2 + b1 + b2)[u] (transposed)
  - V-dot on PE: logits[1, t] = sum_uc V[uc].T @ score[uc]
  - softmax per batch in [1, T] row layout (DVE max, ACT exp w/ accum sum)
  - attn row -> 16 tiny PE transposes -> [128, 16] chunk layout
  - context: psum[1, D] = sum_c attn[:, c].T @ Fn[c]  (float32r)
"""

from contextlib import ExitStack

import numpy as np

import concourse.bass as bass
import concourse.tile as tile
from concourse import bacc, mybir
from concourse.bass_utils import run_bass_kernel_spmd
from concourse.masks import make_identity

f32 = mybir.dt.float32
f32r = mybir.dt.float32r
Tanh = mybir.ActivationFunctionType.Tanh
Exp = mybir.ActivationFunctionType.Exp

P = 128
B, T, D, U = 32, 2048, 512, 512
NCORES = 8
BL = B // NCORES  # 4 local batches
KC = D // P       # 4 contraction chunks
UC = U // P       # 4 u chunks
TGC = 512         # t columns per group
G = T // TGC      # 4 groups per batch
TT = TGC // P     # 4 t-subtiles per group
NCH = T // P      # 16 t-chunks per batch


def _r(ap):
    """float32r view (full-rate fp32 matmul path on TRN2)."""
    return ap.bitcast(f32r)


def build_kernel():
    nc = bacc.Bacc("TRN2", target_bir_lowering=False, debug=False,
                   num_devices=NCORES)

    # features/W1/V feed float32r matmuls; declaring them float32r end-to-end
    # satisfies walrus's "producer must round to FP32r" verification (bytes
    # are plain fp32 either way).
    feat = nc.dram_tensor("features", [BL, T, D], f32r, kind="ExternalInput").ap()
    hid = nc.dram_tensor("hidden", [BL, D], f32, kind="ExternalInput").ap()
    w1 = nc.dram_tensor("W1", [D, U], f32r, kind="ExternalInput").ap()
    b1 = nc.dram_tensor("b1", [U], f32, kind="ExternalInput").ap()
    w2 = nc.dram_tensor("W2", [D, U], f32, kind="ExternalInput").ap()
    b2 = nc.dram_tensor("b2", [U], f32, kind="ExternalInput").ap()
    v = nc.dram_tensor("V", [U, 1], f32r, kind="ExternalInput").ap()
    ctx_out = nc.dram_tensor("context", [BL, D], f32, kind="ExternalOutput").ap()
    attn_out = nc.dram_tensor("attn", [BL, T, 1], f32, kind="ExternalOutput").ap()

    with tile.TileContext(nc) as tc, ExitStack() as ctx:
        consts = ctx.enter_context(tc.tile_pool(name="consts", bufs=1))
        fn_pool = ctx.enter_context(tc.tile_pool(name="fnp", bufs=32))
        ft_pool = ctx.enter_context(tc.tile_pool(name="ftp", bufs=8))
        sc_pool = ctx.enter_context(tc.tile_pool(name="scp", bufs=8))
        row_pool = ctx.enter_context(tc.tile_pool(name="rowp", bufs=4))
        small = ctx.enter_context(tc.tile_pool(name="smallp", bufs=4))
        ps = ctx.enter_context(tc.tile_pool(name="ps", bufs=2, space="PSUM"))

        # ---------------- constants ----------------
        ident = consts.tile([P, P], f32)
        make_identity(nc, ident)
        identr = consts.tile([P, P], f32r)
        nc.vector.tensor_copy(out=identr, in_=ident)

        w1_sb = consts.tile([P, KC, U], f32r)
        nc.sync.dma_start(out=w1_sb, in_=w1.rearrange("(k p) u -> p k u", p=P))
        w2_sb = consts.tile([P, KC, U], f32)
        nc.sync.dma_start(out=w2_sb, in_=w2.rearrange("(k p) u -> p k u", p=P))

        hid_nat = consts.tile([BL, D], f32)
        nc.sync.dma_start(out=hid_nat, in_=hid)

        b1_sb = consts.tile([1, U], f32)
        nc.sync.dma_start(out=b1_sb, in_=b1.rearrange("(o u) -> o u", o=1))
        b2_sb = consts.tile([1, U], f32)
        nc.sync.dma_start(out=b2_sb, in_=b2.rearrange("(o u) -> o u", o=1))
        bsum = consts.tile([1, U], f32)
        nc.vector.tensor_add(bsum, b1_sb, b2_sb)

        v_sb = consts.tile([P, UC], f32r)
        nc.sync.dma_start(out=v_sb, in_=v.rearrange("(k p) o -> p (k o)", p=P))

        ones_bl = consts.tile([1, BL], f32)
        nc.vector.memset(ones_bl, 1.0)

        # hidden transposed: hidT[p, k, b] = hidden[b, k*P + p]
        hidT = consts.tile([P, KC, BL], f32)
        hps = ps.tile([P, KC * BL], f32, tag="small", bufs=2, name="hps")
        for k in range(KC):
            nc.tensor.transpose(hps[:, k * BL:(k + 1) * BL],
                                hid_nat[:, k * P:(k + 1) * P],
                                ident[0:BL, 0:BL])
        nc.vector.tensor_copy(out=hidT.rearrange("p k b -> p (k b)"), in_=hps)

        # combined per-(u, b) bias: combT[u, b] = (hidden @ W2)[b, u] + b1[u] + b2[u]
        combT = consts.tile([P, UC, BL], f32)
        for uc in range(UC):
            cps = ps.tile([P, BL], f32, tag="small", bufs=2, name="cps")
            for k in range(KC):
                nc.tensor.matmul(cps, lhsT=w2_sb[:, k, uc * P:(uc + 1) * P],
                                 rhs=hidT[:, k, :],
                                 start=(k == 0), stop=False)
            nc.tensor.matmul(cps, lhsT=bsum[0:1, uc * P:(uc + 1) * P],
                             rhs=ones_bl, start=False, stop=True)
            nc.vector.tensor_copy(out=combT[:, uc, :], in_=cps)

        # ---------------- main pipeline ----------------
        fn_tiles = {}

        def load_fn(b):
            for ch in range(NCH):
                fnt = fn_pool.tile([P, D], f32r, tag="fn", name="fnt")
                nc.sync.dma_start(out=fnt, in_=feat[b, ch * P:(ch + 1) * P, :])
                fn_tiles[(b, ch)] = fnt

        def do_group(b, g, logits_row):
            # transpose F for this group: FT[k] is [128 d, TGC t]
            fts = []
            for k in range(KC):
                ftp = ps.tile([P, TGC], f32r, tag="ft", bufs=2, name="ftp")
                for tt in range(TT):
                    fnt = fn_tiles[(b, g * TT + tt)]
                    nc.tensor.transpose(ftp[:, tt * P:(tt + 1) * P],
                                        fnt[:, k * P:(k + 1) * P], identr)
                ft_sb = ft_pool.tile([P, TGC], f32r, tag="ft", name="ft_sb")
                nc.vector.tensor_copy(out=ft_sb, in_=ftp)
                fts.append(ft_sb)

            # main matmul + tanh, u-chunk at a time; score kept for V-dot
            scs = []
            for uc in range(UC):
                mmp = ps.tile([P, TGC], f32, tag="mm", bufs=2, name="mmp")
                for k in range(KC):
                    nc.tensor.matmul(mmp,
                                     lhsT=w1_sb[:, k, uc * P:(uc + 1) * P],
                                     rhs=fts[k],
                                     start=(k == 0), stop=(k == KC - 1))
                sct = sc_pool.tile([P, TGC], f32r, tag="sc", name="sct")
                nc.scalar.activation(out=sct, in_=mmp, func=Tanh,
                                     bias=combT[:, uc, b:b + 1], scale=1.0)
                scs.append(sct)

            # V-dot: logits [1, TGC] = sum_uc V[uc].T @ score[uc]
            vp = ps.tile([1, TGC], f32, tag="vd", bufs=1, name="vp")
            for uc in range(UC):
                nc.tensor.matmul(vp, lhsT=v_sb[:, uc:uc + 1], rhs=scs[uc],
                                 start=(uc == 0), stop=(uc == UC - 1))
            nc.scalar.copy(out=logits_row[0:1, g * TGC:(g + 1) * TGC], in_=vp)

        def finish_batch(b, logits_row):
            gmax = small.tile([1, 1], f32, tag="g1", name="gmax")
            nc.vector.tensor_reduce(gmax, logits_row, axis=mybir.AxisListType.X,
                                    op=mybir.AluOpType.max)
            negm = small.tile([1, 1], f32, tag="g1", name="negm")
            nc.vector.tensor_scalar_mul(negm, gmax, -1.0)

            exps = row_pool.tile([1, T], f32, tag="row", name="exps")
            gsum = small.tile([1, 1], f32, tag="g1", name="gsum")
            nc.scalar.activation(out=exps, in_=logits_row, func=Exp,
                                 bias=negm, scale=1.0, accum_out=gsum)
            rs = small.tile([1, 1], f32, tag="g1", name="rs")
            nc.vector.reciprocal(rs, gsum)

            attn_row = row_pool.tile([1, T], f32, tag="row", name="attn_row")
            nc.scalar.mul(out=attn_row, in_=exps, mul=rs)
            nc.sync.dma_start(
                out=attn_out[b].rearrange("(o t) one -> o (t one)", o=1),
                in_=attn_row)

            # reshape attn row -> [128, 16] (chunk c in column c)
            app = ps.tile([P, NCH], f32, tag="small", bufs=2, name="app")
            for c in range(NCH):
                nc.tensor.transpose(app[:, c:c + 1],
                                    attn_row[0:1, c * P:(c + 1) * P],
                                    ident[0:1, 0:1])
            attn_pt = small.tile([P, NCH], f32r, tag="apt", name="attn_pt")
            nc.vector.tensor_copy(out=attn_pt, in_=app)

            # context: [1, D] = sum_c attn[:, c].T @ Fn[c]
            cxp = ps.tile([1, D], f32, tag="cx", bufs=1, name="cxp")
            for c in range(NCH):
                nc.tensor.matmul(cxp, lhsT=attn_pt[:, c:c + 1],
                                 rhs=fn_tiles[(b, c)],
                                 start=(c == 0), stop=(c == NCH - 1))
            ctx_sb = small.tile([1, D], f32, tag="cxs", name="ctx_sb")
            nc.scalar.copy(out=ctx_sb, in_=cxp)
            nc.sync.dma_start(out=ctx_out[b:b + 1, :], in_=ctx_sb)

        load_fn(0)
        for b in range(BL):
            logits_row = row_pool.tile([1, T], f32, tag="row", name="logits_row")
            for g in range(G):
                do_group(b, g, logits_row)
                if g == 0 and b + 1 < BL:
                    load_fn(b + 1)
            finish_batch(b, logits_row)

    nc.compile()
    return nc


_NC_CACHE = None


def _get_nc():
    global _NC_CACHE
    if _NC_CACHE is None:
        _NC_CACHE = build_kernel()
    return _NC_CACHE


def _as_np(x):
    return np.ascontiguousarray(np.asarray(x, dtype=np.float32))


def make_in_maps(features, hidden, W1, b1, W2, b2, V):
    features = _as_np(features)
    hidden = _as_np(hidden)
    W1, b1, W2, b2, V = map(_as_np, (W1, b1, W2, b2, V))
    in_maps = []
    for c in range(NCORES):
        sl = slice(c * BL, (c + 1) * BL)
        in_maps.append({
            "features": features[sl],
            "hidden": hidden[sl],
            "W1": W1, "b1": b1, "W2": W2, "b2": b2, "V": V,
        })
    return in_maps


def kernel(features, hidden, W1, b1, W2, b2, V, bv):
    # bv shifts logits by a constant; softmax is shift-invariant and bv does
    # not appear in either output, so it is unused.
    nc = _get_nc()
    in_maps = make_in_maps(features, hidden, W1, b1, W2, b2, V)
    res = run_bass_kernel_spmd(nc, in_maps, core_ids=list(range(NCORES)))
    context = np.concatenate([res.results[c]["context"] for c in range(NCORES)],
                             axis=0)
    attn = np.concatenate([res.results[c]["attn"] for c in range(NCORES)],
                          axis=0)
    return context, attn
